# revision 1
# baseline (speedup 1.0000x reference)
"""Conv4d (kernel 3^4, circular, grouped-over-time) on 8 TRN2 NeuronCores.

Math: res[b,co,t] = sum_g conv3d_valid(pad_wrap1(x[b,:,s=t-1+g]), W[g]) + bias,
with s circular over the 16 time slices.

Device scheme (per core = one (batch, 8-time-slice) shard):
  - outputs processed in pairs (t, t+1); PSUM partitions = (t-sel u, c_out)
  - contraction K = (input-slice-sel j, c_in) over pair-tiles of two
    consecutive padded slices stacked on partitions
  - per (kd,kh,kw) tap and output pair: 2 matmuls, K=128 M=128 N=512 fp32r:
      L-block: slices (t-1, t),  g = j - u      (g=-1 entry zeroed)
      H-block: slices (t+1, t+2), g = j - u + 2 (g=3 entry zeroed)
  - rhs = 3-level-AP window into the padded 18^3 slice cube, 2 output
    d-planes per matmul -> 8 PSUM banks cover the 16^3 spatial output
  - bias added during PSUM->SBUF evacuation (DVE tensor_scalar_add)
"""
import numpy as np

B, C, S, KW = 4, 64, 16, 3
SP = S + 2          # padded spatial extent
CUBE = SP * SP * SP  # 5832 padded elements per channel
NCORES = 8
TSH = S * B // NCORES  # 8 output time slices per core

_PROGRAM = None


def _build_program():
    import concourse.bacc as bacc
    import concourse.mybir as mybir
    import concourse.tile as tile

    nc = bacc.Bacc("TRN2", target_bir_lowering=False, debug=False,
                   num_devices=NCORES)
    f32r = mybir.dt.float32r
    f32 = mybir.dt.float32

    xs_d = nc.dram_tensor("xs", [5, 128, CUBE], f32r, kind="ExternalInput").ap()
    wl_d = nc.dram_tensor("wl", [128, 27 * 128], f32r, kind="ExternalInput").ap()
    wh_d = nc.dram_tensor("wh", [128, 27 * 128], f32r, kind="ExternalInput").ap()
    bias_d = nc.dram_tensor("bias2", [128, 1], f32, kind="ExternalInput").ap()
    y_d = nc.dram_tensor("y", [TSH, C, S * S * S], f32, kind="ExternalOutput").ap()

    with tile.TileContext(nc) as tc:
        with (
            tc.tile_pool(name="xp", bufs=1) as xpool,
            tc.tile_pool(name="wp", bufs=1) as wpool,
            tc.tile_pool(name="st", bufs=2) as spool,
            tc.tile_pool(name="ps", bufs=8, space="PSUM") as pspool,
        ):
            # Issue order matters: the first matmuls (pair 0, chunk 0) need
            # only the first tap-group of weights and the first plane-group
            # of xt0/xt1.  Interleave small pieces, weights on the gpsimd
            # DMA queue and x on the sync queue so they stream in parallel.
            wlt = wpool.tile([128, 27 * 128], f32r)
            wht = wpool.tile([128, 27 * 128], f32r)
            bias_t = wpool.tile([128, 1], f32)
            xts = []
            for k in range(5):
                xt = xpool.tile([128, CUBE], f32r, name=f"xt{k}")
                xts.append(xt)
            piece = 6 * SP * SP  # 6 d-planes
            wpiece = 7 * 128     # 7 taps of weights

            def wdma(p):
                lo, hi = p * wpiece, min((p + 1) * wpiece, 27 * 128)
                nc.gpsimd.dma_start(wlt[:, lo:hi], wl_d[:, lo:hi])
                nc.gpsimd.dma_start(wht[:, lo:hi], wh_d[:, lo:hi])

            def xdma(k, p):
                nc.sync.dma_start(
                    xts[k][:, p * piece:(p + 1) * piece],
                    xs_d[k][:, p * piece:(p + 1) * piece],
                )

            wdma(0)
            xdma(0, 0)
            xdma(1, 0)
            nc.gpsimd.dma_start(bias_t[:], bias_d)
            wdma(1)
            xdma(0, 1)
            xdma(1, 1)
            wdma(2)
            xdma(0, 2)
            xdma(1, 2)
            wdma(3)
            for k in (2, 3, 4):
                for p in range(3):
                    xdma(k, p)

            xvs = [xt.rearrange("p (d h w) -> p d h w", d=SP, h=SP, w=SP)
                   for xt in xts]

            for u in range(TSH // 2):  # output pair
                stage = spool.tile([128, S * S * S], f32, name="stage")
                for c in range(8):  # 2 output d-planes per chunk
                    bank = pspool.tile([128, 512], f32, name="bank")
                    nmm = 0
                    for kd in range(KW):
                        for kh in range(KW):
                            for kw in range(KW):
                                i = (kd * KW + kh) * KW + kw
                                for wt, xv in ((wlt, xvs[u]), (wht, xvs[u + 1])):
                                    rhs = xv[:, 2 * c + kd:2 * c + kd + 2,
                                             kh:kh + S, kw:kw + S]
                                    nc.tensor.matmul(
                                        bank[:],
                                        wt[:, i * 128:(i + 1) * 128],
                                        rhs,
                                        start=(nmm == 0), stop=(nmm == 53),
                                    )
                                    nmm += 1
                    nc.vector.tensor_scalar_add(
                        stage[:, c * 512:(c + 1) * 512], bank[:], bias_t[:]
                    )
                    nc.sync.dma_start(
                        y_d[2 * u][:, c * 512:(c + 1) * 512],
                        stage[0:C, c * 512:(c + 1) * 512],
                    )
                    nc.sync.dma_start(
                        y_d[2 * u + 1][:, c * 512:(c + 1) * 512],
                        stage[C:128, c * 512:(c + 1) * 512],
                    )

    nc.compile()
    return nc


def _host_prep(x, weight, bias):
    """Build per-core input maps."""
    # padded slices: xp[b, s] = wrap-pad1 of x[b,:,s] -> (C, 18,18,18)
    xpad = np.pad(x, ((0, 0), (0, 0), (0, 0), (1, 1), (1, 1), (1, 1)),
                  mode="wrap").astype(np.float32)  # (B, C, S, 18,18,18)

    # weight block-banded lhsT tiles: [128=(j,ci), 27*128=(tap,(u,co))]
    wl = np.zeros((128, 27, 128), dtype=np.float32)
    wh = np.zeros((128, 27, 128), dtype=np.float32)
    for kd in range(KW):
        for kh in range(KW):
            for kw in range(KW):
                i = (kd * KW + kh) * KW + kw
                for j in range(2):
                    for u in range(2):
                        gl = j - u
                        if 0 <= gl < KW:
                            wl[j * C:(j + 1) * C, i, u * C:(u + 1) * C] = \
                                weight[gl, :, :, kd, kh, kw].T
                        gh = j - u + 2
                        if 0 <= gh < KW:
                            wh[j * C:(j + 1) * C, i, u * C:(u + 1) * C] = \
                                weight[gh, :, :, kd, kh, kw].T
    wl = wl.reshape(128, 27 * 128)
    wh = wh.reshape(128, 27 * 128)
    bias2 = np.concatenate([bias, bias]).astype(np.float32).reshape(128, 1)

    in_maps = []
    for core in range(NCORES):
        b = core // 2
        t0 = TSH * (core % 2)
        xs = np.empty((5, 128, CUBE), dtype=np.float32)
        for k in range(5):
            sa = (t0 - 1 + 2 * k) % S
            sb = (t0 + 2 * k) % S
            xs[k, 0:C] = xpad[b, :, sa].reshape(C, CUBE)
            xs[k, C:128] = xpad[b, :, sb].reshape(C, CUBE)
        in_maps.append({"xs": xs, "wl": wl, "wh": wh, "bias2": bias2})
    return in_maps


LAST_RESULTS = None


def kernel(x, weight, bias, _trace=False):
    global _PROGRAM, LAST_RESULTS
    from concourse import bass_utils

    x = np.asarray(x, dtype=np.float32)
    weight = np.asarray(weight, dtype=np.float32)
    bias = np.asarray(bias, dtype=np.float32)

    if _PROGRAM is None:
        _PROGRAM = _build_program()
    nc = _PROGRAM

    in_maps = _host_prep(x, weight, bias)
    res = bass_utils.run_bass_kernel_spmd(
        nc, in_maps, core_ids=list(range(NCORES)), trace=_trace
    )
    LAST_RESULTS = res

    out = np.empty((B, C, S, S, S, S), dtype=np.float32)
    for core in range(NCORES):
        b = core // 2
        t0 = TSH * (core % 2)
        y = res.results[core]["y"]  # (TSH, C, 4096)
        out[b, :, t0:t0 + TSH] = y.transpose(1, 0, 2).reshape(C, TSH, S, S, S)
    return out



# revision 8
# speedup vs baseline: 1.7292x; 1.7292x over previous
"""Conv4d (kernel 3^4, circular, grouped-over-time) on 8 TRN2 NeuronCores.

Math: res[b,co,t] = sum_g conv3d_valid(pad_wrap1(x[b,:,s=t-1+g]), W[g]) + bias,
with s circular over the 16 time slices.

v2: Winograd F(2,3) along the w axis + bf16 matmuls.
  - host transforms x -> D (4 m-components per 2-output w-tile, 8 tiles
    across the 18-wide padded w axis) and W -> Wtilde (kw -> m), so the
    27 spatial taps become 36 m-taps whose columns each serve TWO
    outputs: PE column count drops 1/3 vs direct conv.
  - device scheme per core (= one (batch, 8-time-slice) shard):
    outputs in pairs (t, t+1); PSUM partitions = (t-sel u, c_out);
    contraction K = (input-slice-sel j, c_in) over pair-tiles of two
    consecutive w-transformed slices (L: (t-1,t), H: (t+1,t+2)),
    block-banded weights zero where g = j-u (+2) is outside 0..2.
  - per output pair, 4 chunks of 4 d-planes: for each m in 0..3
    accumulate m~[m] (18 matmuls: 3 kd x 3 kh x {L,H}, N=512 bf16)
    into a quarter of a 4-bank PSUM tile; DVE combines
      even w: (m0+m1)+m2 + bias,  odd w: (m1-m2)+bias - m3
    via 2 tensor_tensor + 2 scalar_tensor_tensor ops, writing the
    interleaved-w stage tile evacuated by 2 output DMAs.
"""
import numpy as np
import ml_dtypes

B, C, S, KW = 4, 64, 16, 3
SP = S + 2           # padded spatial extent (d, h)
WT = 8               # w tiles per slice (F(2,3): 2 outputs per tile)
M4 = 4               # Winograd components per tile
DROW = SP * WT * M4  # 576 elems per (d) plane per partition
DCUBE = SP * DROW    # 10368 elems per transformed slice per partition
NTAP = 9 * M4        # 36 m-taps
NCORES = 8
TSH = S * B // NCORES  # 8 output time slices per core

_PROGRAM = None


def _build_program():
    import concourse.bacc as bacc
    import concourse.mybir as mybir
    import concourse.tile as tile

    nc = bacc.Bacc("TRN2", target_bir_lowering=False, debug=False,
                   num_devices=NCORES)
    bf16 = mybir.dt.bfloat16
    f32 = mybir.dt.float32

    xs_d = nc.dram_tensor("xs", [5, 128, DCUBE], bf16, kind="ExternalInput").ap()
    wl_d = nc.dram_tensor("wl", [128, NTAP * 128], bf16, kind="ExternalInput").ap()
    wh_d = nc.dram_tensor("wh", [128, NTAP * 128], bf16, kind="ExternalInput").ap()
    bias_d = nc.dram_tensor("bias2", [128, 1], f32, kind="ExternalInput").ap()
    y_d = nc.dram_tensor("y", [TSH, C, S * S * S], f32, kind="ExternalOutput").ap()

    with tile.TileContext(nc) as tc:
        with (
            tc.tile_pool(name="xp", bufs=1) as xpool,
            tc.tile_pool(name="wp", bufs=1) as wpool,
            tc.tile_pool(name="st", bufs=4) as spool,
            tc.tile_pool(name="tp", bufs=4) as tpool,
            tc.tile_pool(name="ps", bufs=2, space="PSUM") as pspool,
        ):
            wlt = wpool.tile([128, NTAP * 128], bf16)
            wht = wpool.tile([128, NTAP * 128], bf16)
            bias_t = wpool.tile([128, 1], f32)
            xts = [xpool.tile([128, DCUBE], bf16, name=f"xt{k}")
                   for k in range(5)]
            wpiece = 9 * 128     # one m-group of taps per DMA piece

            def wdma(p):
                lo, hi = p * wpiece, (p + 1) * wpiece
                nc.gpsimd.dma_start(wlt[:, lo:hi], wl_d[:, lo:hi])
                nc.gpsimd.dma_start(wht[:, lo:hi], wh_d[:, lo:hi])

            xs_dv = [xs_d[k].rearrange("p (m d r) -> p m d r",
                                       m=M4, d=SP, r=SP * WT)
                     for k in range(5)]
            xts_v = [xt.rearrange("p (m d r) -> p m d r",
                                  m=M4, d=SP, r=SP * WT)
                     for xt in xts]

            def xdma(k, p):
                nc.sync.dma_start(
                    xts_v[k][:, :, p * 6:(p + 1) * 6, :],
                    xs_dv[k][:, :, p * 6:(p + 1) * 6, :],
                )

            wdma(0)
            xdma(0, 0)
            xdma(1, 0)
            nc.gpsimd.dma_start(bias_t[:], bias_d)
            wdma(1)
            xdma(0, 1)
            xdma(1, 1)
            wdma(2)
            xdma(0, 2)
            xdma(1, 2)
            wdma(3)
            for k in (2, 3, 4):
                for p in range(3):
                    xdma(k, p)

            xvs = [xt.rearrange("p (m d h t) -> p m d h t",
                                m=M4, d=SP, h=SP, t=WT)
                   for xt in xts]

            add = mybir.AluOpType.add
            sub = mybir.AluOpType.subtract

            for u in range(TSH // 2):    # output pair
                for c in range(4):       # 4 output d-planes per chunk
                    bank = pspool.tile([128, 4 * 512], f32, name="bank")
                    for m in range(M4):
                        reg = bank[:, m * 512:(m + 1) * 512]
                        nmm = 0
                        for kd in range(KW):
                            for kh in range(KW):
                                i = (m * KW + kd) * KW + kh
                                for wt, xv in ((wlt, xvs[u]), (wht, xvs[u + 1])):
                                    rhs = xv[:, m, 4 * c + kd:4 * c + kd + 4,
                                             kh:kh + S, :]
                                    nc.tensor.matmul(
                                        reg,
                                        wt[:, i * 128:(i + 1) * 128],
                                        rhs,
                                        start=(nmm == 0), stop=(nmm == 17),
                                    )
                                    nmm += 1
                    stage = spool.tile([128, 1024], f32, name="stage")
                    sv = stage.rearrange("p (d h w) -> p d h w",
                                         d=4, h=S, w=S)
                    # DVE reads at most ONE PSUM operand per op (1 rd port):
                    #   e1 = m1 + bias; even = (e1+m0)+m2; odd = (e1-m3)-m2
                    e1 = tpool.tile([128, 512], f32, name="e1")
                    ta = tpool.tile([128, 512], f32, name="ta")
                    tb = tpool.tile([128, 512], f32, name="tb")
                    m0 = bank[:, 0:512]
                    m1 = bank[:, 512:1024]
                    m2 = bank[:, 1024:1536]
                    m3 = bank[:, 1536:2048]
                    nc.vector.tensor_scalar_add(e1[:], m1, bias_t[:])
                    nc.vector.tensor_tensor(ta[:], e1[:], m0, add)
                    nc.vector.tensor_tensor(sv[:, :, :, 0:S:2], ta[:], m2, add)
                    nc.vector.tensor_tensor(tb[:], e1[:], m3, sub)
                    nc.vector.tensor_tensor(sv[:, :, :, 1:S:2], tb[:], m2, sub)
                    nc.sync.dma_start(
                        y_d[2 * u][:, c * 1024:(c + 1) * 1024],
                        stage[0:C, :],
                    )
                    nc.sync.dma_start(
                        y_d[2 * u + 1][:, c * 1024:(c + 1) * 1024],
                        stage[C:128, :],
                    )

    nc.compile()
    return nc


def _host_prep(x, weight, bias):
    """Build per-core input maps (w-Winograd-transformed, bf16)."""
    xpad = np.pad(x, ((0, 0), (0, 0), (0, 0), (1, 1), (1, 1), (1, 1)),
                  mode="wrap").astype(np.float32)  # (B, C, S, 18,18,18)
    # w-axis input transform: D[..., tile, m], tiles at w = 2*tile
    a0 = xpad[..., 0:16:2]
    a1 = xpad[..., 1:17:2]
    a2 = xpad[..., 2:18:2]
    a3 = xpad[..., 3::2]
    dt = np.stack([a0 - a2, a1 + a2, a2 - a1, a1 - a3], axis=3)
    dt = np.ascontiguousarray(dt, dtype=ml_dtypes.bfloat16)
    dt = dt.reshape(B, C, S, DCUBE)  # (b, ci, s, (m d h tile))

    # weight transform along kw -> m
    w0, w1, w2 = weight[..., 0], weight[..., 1], weight[..., 2]
    wt = np.stack([w0, 0.5 * (w0 + w1 + w2), 0.5 * (w0 - w1 + w2), w2],
                  axis=-1)  # (K, co, ci, kd, kh, m)

    # block-banded lhsT tiles: [128=(j,ci), NTAP*128=((m,kd,kh),(u,co))]
    wl = np.zeros((128, NTAP, 128), dtype=np.float32)
    wh = np.zeros((128, NTAP, 128), dtype=np.float32)
    for m in range(M4):
        for kd in range(KW):
            for kh in range(KW):
                i = (m * KW + kd) * KW + kh
                for j in range(2):
                    for u in range(2):
                        gl = j - u
                        if 0 <= gl < KW:
                            wl[j * C:(j + 1) * C, i, u * C:(u + 1) * C] = \
                                wt[gl, :, :, kd, kh, m].T
                        gh = j - u + 2
                        if 0 <= gh < KW:
                            wh[j * C:(j + 1) * C, i, u * C:(u + 1) * C] = \
                                wt[gh, :, :, kd, kh, m].T
    wl = wl.reshape(128, NTAP * 128).astype(ml_dtypes.bfloat16)
    wh = wh.reshape(128, NTAP * 128).astype(ml_dtypes.bfloat16)
    bias2 = np.concatenate([bias, bias]).astype(np.float32).reshape(128, 1)

    in_maps = []
    for core in range(NCORES):
        b = core // 2
        t0 = TSH * (core % 2)
        xs = np.empty((5, 128, DCUBE), dtype=ml_dtypes.bfloat16)
        for k in range(5):
            sa = (t0 - 1 + 2 * k) % S
            sb = (t0 + 2 * k) % S
            xs[k, 0:C] = dt[b, :, sa]
            xs[k, C:128] = dt[b, :, sb]
        in_maps.append({"xs": xs, "wl": wl, "wh": wh, "bias2": bias2})
    return in_maps


LAST_RESULTS = None


def kernel(x, weight, bias, _trace=False):
    global _PROGRAM, LAST_RESULTS
    from concourse import bass_utils

    x = np.asarray(x, dtype=np.float32)
    weight = np.asarray(weight, dtype=np.float32)
    bias = np.asarray(bias, dtype=np.float32)

    if _PROGRAM is None:
        _PROGRAM = _build_program()
    nc = _PROGRAM

    in_maps = _host_prep(x, weight, bias)
    res = bass_utils.run_bass_kernel_spmd(
        nc, in_maps, core_ids=list(range(NCORES)), trace=_trace
    )
    LAST_RESULTS = res

    out = np.empty((B, C, S, S, S, S), dtype=np.float32)
    for core in range(NCORES):
        b = core // 2
        t0 = TSH * (core % 2)
        y = res.results[core]["y"]  # (TSH, C, 4096)
        out[b, :, t0:t0 + TSH] = y.transpose(1, 0, 2).reshape(C, TSH, S, S, S)
    return out
